# revision 8
# baseline (speedup 1.0000x reference)
"""Trainium2 Bass kernel for per-voxel 3x3 SPD matrix logarithm (v2.3).

Input  x: (2, 9, 64, 128, 128) fp32, channel c = 3*i+j of symmetric M.
Output Y: same shape, Y = U log(S) U^T per voxel.

Design:
  - fp16 tiles for DVE tensor_tensor/tensor_scalar work (2x/4x DVE perf
    modes); r-chain fp32 (custom/ACT ops are dtype-blind so fp32 is free).
  - FD=1024, NCHUNK=2 (one chunk per batch); all emission interleaved
    chunk0/chunk1 op-by-op so the two serial chains hide each other's
    cross-engine latency. 3 ACT table loads total (sqrt -> trig -> ln).
  - Fused a/b divides + reciprocal custom DVE ops (bit-trick NR).
  - Engine split tuned for both balance and latency: det-critical ops on
    DVE (fast), Pool takes off-critical work (ad, cross, w3, Yd head).
  - Only 6 output channels stored; symmetric duplicates replicated on host.
  - SBUF fits via tile aliasing (tags hold several sequential lives).

Math (branchless, eigenvector-free):
  q = tr(M)/3, D = M - q I, p = sqrt(tr(D^2)/6), r = det(D)/(2 p^3)
  phi = acos(clamp(r))/3  (via arctan + sin on the ACT engine)
  lam = q + 2p cos(phi + {0, -2pi/3, +2pi/3})  (ascending: l1,l2,l3)
  c1 = f[l1,l2], c2 = f[l1,l2,l3] (divided differences of ln)
  sigma = c1/c2 + 2 p cos(phi); gamma = ln(l1) + u*(c1 + c2*(2 pc - u))
  log(M) = c2 * (D*(D + sigma I) + w3) + gamma I  (diag; w3_i = su - sqU_rev_i)
  offdiag = c2 * (bce*(sigma - D_rev) + cross),   cross = (ce, be, bc)
"""
import math
import numpy as np

import concourse.bacc as bacc
import concourse.tile as tile
import concourse.bass as bass
from concourse import mybir
from concourse.bass_utils import run_bass_kernel_spmd

F32 = mybir.dt.float32
F16 = mybir.dt.float16
OP = mybir.AluOpType
AF = mybir.ActivationFunctionType

B = 2
NV = 64 * 128 * 128
NCORE = 8
VPC = NV // NCORE            # 131072 voxels per core
P = 128
FD = 1024
CPB = VPC // (P * FD)        # 1 chunk per batch
NCHUNK = B * CPB             # 2
PLANE = VPC // P             # 1024

DT = F16                     # main compute dtype
CL = 0.9995                  # r clamp (fp16-safe: nextafter stays < 1)
S3 = math.sqrt(3.0)
PI6 = math.pi / 6.0
ETA = 1e-4                   # eigen-gap floor (fp16-safe)

# ---- runtime-registered custom DVE ops ----
from concourse import dve_ops as _dvo
from concourse.dve_spec import (
    Spec as _Spec, Src0 as _S0, Src1 as _S1, C0 as _C0, C1 as _C1,
    C2 as _C2, maxx as _maxx, minn as _minn, lower as _lower,
    _has_src1 as _hs1, Bin as _Bin, AluOp as _AluOp,
)
from concourse.dve_uop import DveOpSpec as _DveOpSpec
from concourse.dve_ops import RECIPROCAL_APPROX_FAST, RECIP_APPROX_FAST_CONSTS


def _register_dve(name, spec):
    if name in _dvo._SUB_OPCODE_FOR_NAME:
        return next(op for op in _dvo.OPS if op.name == name)
    op = _dvo.DveOp(name, spec, subdim=False, uops_sha={})
    _dvo.OPS.append(op)
    _dvo.CUSTOM_DVE_SPECS[name] = spec
    row = _dvo._CUSTOM_DVE_ROW_BASE + len(_dvo.OPS) - 1
    assert row < 0x20
    _dvo._SUB_OPCODE_FOR_NAME[name] = row
    for ver in ("v3", "v4"):
        uops = _lower(spec, ver=ver)
        res = _DveOpSpec(name=name, opcode=row, uops=uops, rd1_en=_hs1(spec))
        op.uops_sha[ver] = res.sha(ver)
    return op


# r = clamp(in0 * in1^3 * s0, s1, imm2)
DETC_CLAMP = _register_dve("LOGM_DETC_CLAMP", _Spec(
    body=_minn(_maxx(_S0 * (_S1 * _S1 * _S1) * _C0, _C1), _C2),
    reference=lambda in0, in1, s0, s1, imm2: np.minimum(
        np.maximum(in0.astype(np.float32) * (in1.astype(np.float32) ** 3) * s0, s1), imm2
    ).astype(np.float32),
))
# d23 = max(in0*s0 - in1, s1)
SCALE_SUBMAX = _register_dve("LOGM_SCALE_SUBMAX", _Spec(
    body=_maxx(_S0 * _C0 - _S1, _C1),
    reference=lambda in0, in1, s0, s1, imm2: np.maximum(
        in0.astype(np.float32) * s0 - in1, s1
    ).astype(np.float32),
))

# Fused divide out = in0 / in1 via 1-Newton bit-trick reciprocal.
# z = x*bitcast(~x) lands in [-4.5,-4]; y1 = y0*(c1 - x*y0), y0 = ~x * c0.
# Minimax consts for 1 NR pass over [-4.5,-4]: rel err <= ~1.8e-3.
_D_C0 = -0.235434
_D_C1 = 2.001415
_dnx = _Bin(_AluOp.BITWISE_NOT, _S1, _S1)
_dy0 = _dnx * _C0
_dy1 = _dy0 * (_C1 - _S1 * _dy0)


def _ref_div1(in0, in1, s0, s1, imm2):
    x = in1.astype(np.float32)
    not_x = (~x.view(np.int32)).view(np.float32)
    y0 = not_x * np.float32(s0)
    y1 = y0 * (np.float32(s1) - x * y0)
    return (in0.astype(np.float32) * y1).astype(np.float32)


DIV1 = _register_dve("LOGM_DIV1", _Spec(
    body=_S0 * _dy1,
    reference=_ref_div1,
))

# Clamped divide: out = in0 / max(in1, imm2) -- folds the eigen-gap floor
# into the divide so the d-gap ops need no separate max.
_S1c = _maxx(_S1, _C2)
_dnxc = _Bin(_AluOp.BITWISE_NOT, _S1c, _S1c)
_dy0c = _dnxc * _C0
_dy1c = _dy0c * (_C1 - _S1c * _dy0c)


def _ref_div1m(in0, in1, s0, s1, imm2):
    x = np.maximum(in1.astype(np.float32), np.float32(imm2))
    not_x = (~x.view(np.int32)).view(np.float32)
    y0 = not_x * np.float32(s0)
    y1 = y0 * (np.float32(s1) - x * y0)
    return (in0.astype(np.float32) * y1).astype(np.float32)


DIV1M = _register_dve("LOGM_DIV1M", _Spec(
    body=_S0 * _dy1c,
    reference=_ref_div1m,
))

# Force Arctan into the same ACT table set as Sin (trig_and_small), matching
# the real cayman act_info.json, so at->sin->sin needs one table load.
from concourse import hw_specs as _hw
import concourse.bacc as _bacc_mod
_orig_gat = _hw.get_activation_tables


def _patched_gat(arch):
    t = _orig_gat(arch)
    for sname, fns in t.items():
        if sname != "trig_and_small":
            fns.discard(mybir.ActivationFunctionType.Arctan)
    return t


_hw.get_activation_tables = _patched_gat
_bacc_mod.get_activation_tables = _patched_gat

_CACHE = {}


def _register_const(nc, val):
    t = nc.alloc_sbuf_tensor(f"const-f32-{val}", [128, 1], F32)
    nc.gpsimd.memset(t.ap(), val)
    nc.const_aps.aps[(F32, float(val))] = t.ap()


def _b3(ap_fd):
    return ap_fd.unsqueeze(1).broadcast_to((P, 3, FD))


def build():
    nc = bacc.Bacc("TRN2")
    _register_const(nc, PI6)
    _register_const(nc, PI6 + math.pi / 2.0)
    nc.all_engine_barrier()

    xin = nc.dram_tensor("xin", [B, 6, VPC], DT, kind="ExternalInput")
    yout = nc.dram_tensor("yout", [B, 6, VPC], DT, kind="ExternalOutput")

    V, G, S = nc.vector, nc.gpsimd, nc.scalar

    with tile.TileContext(nc) as tc:
        with tc.tile_pool(name="mp", bufs=1) as pool:

            def T2(units, name, ci, dt=DT):
                return pool.tile([P, units * FD], dt, name=f"{name}{ci}",
                                 tag=f"{name}{ci}", bufs=1)

            def c3(ap):
                return ap.rearrange("p (c f) -> p c f", c=3)

            def c2v(ap):
                return ap.rearrange("p (c f) -> p c f", c=2)

            shared = {}

            def chunk_steps(ci):
                """Whole chunk as an ordered list of emission thunks."""
                b = ci
                st_ = []

                def s(fn):
                    st_.append(fn)

                # ---- tiles (tags note sequential reuse lives) ----
                adfx = T2(6, "adfx", ci)  # (adf|cross) -> (w3/dwd|cross') packed
                bcdt = T2(6, "bcdt", ci)  # (bce | Dt) packed for 6-wide fusion
                s1 = T2(1, "s1", ci)         # s1 -> uut
                q = T2(1, "q", ci)           # q -> gam
                TU = T2(6, "TU", ci)         # (sqD|sqU) -> (L|DD) -> (Yd|Yo)
                stsu = T2(2, "stsu", ci)
                su2 = T2(1, "su2", ci)       # su2 -> pc2
                p2 = T2(1, "p2", ci)         # p2 -> c2s
                pt = T2(1, "pt", ci)
                ip = T2(1, "ip", ci, F32)
                e1 = T2(3, "e1", ci)         # e1 -> (bcep|dm|det) -> bd
                ad = T2(1, "ad", ci)         # ad -> tq
                dets = T2(1, "dets", ci)     # dets -> at
                sfcf = T2(2, "sfcf", ci)     # sfcf -> gg
                pcps = T2(2, "pcps", ci)
                lg = T2(3, "lg", ci)
                c1f = T2(2, "c1f", ci)
                uu = T2(1, "uu", ci)
                c2 = T2(1, "c2", ci)
                sg = T2(1, "sg", ci)
                g1 = T2(1, "g1", ci)

                sqD, sqU = TU[:, 0:3 * FD], TU[:, 3 * FD:6 * FD]
                sqU3 = c3(sqU)
                st, su = stsu[:, 0:FD], stsu[:, FD:2 * FD]
                b_, c_, e_ = (bcdt[:, 0:FD], bcdt[:, FD:2 * FD], bcdt[:, 2 * FD:3 * FD])
                bcep, dm, det = (e1[:, 0:FD], e1[:, FD:2 * FD], e1[:, 2 * FD:3 * FD])
                # r/r2/sq for BOTH chunks share one tile so the scheduler can't
                # run one chunk's trig ops before the other chunk's sqrt ops
                # (avoids ACT table thrash).
                if "rr" not in shared:
                    shared["rr"] = pool.tile([P, 6 * FD], DT, name="rrpair",
                                             tag="rrpair", bufs=1)
                rr = shared["rr"]
                r = rr[:, (0 + ci) * FD:(1 + ci) * FD]
                r2 = rr[:, (2 + ci) * FD:(3 + ci) * FD]
                sq = rr[:, (4 + ci) * FD:(5 + ci) * FD]
                tq, at = ad[:], dets[:]      # second lives
                L, DD = sqD, sqU             # second life of TU
                l1, l2, l3 = L[:, 0:FD], L[:, FD:2 * FD], L[:, 2 * FD:3 * FD]
                d12, d23, d13 = DD[:, 0:FD], DD[:, FD:2 * FD], DD[:, 2 * FD:3 * FD]
                Yd, Yo = L, DD               # third life of TU
                Yd3, Yo3 = c3(Yd), c3(Yo)
                ps, pc = pcps[:, 0:FD], pcps[:, FD:2 * FD]
                uut = s1                     # second life
                pc2 = su2
                c2s = p2[:]
                gg = sfcf
                gam = q
                Dt3 = c3(bcdt[:, 3 * FD:6 * FD])

                # ---- loads ----
                def loads():
                    nc.sync.dma_start(
                        c2v(adfx[:, 0:2 * FD]),
                        bass.AP(xin, b * 6 * VPC, [[PLANE, P], [VPC, 2], [1, FD]]))
                    nc.sync.dma_start(
                        adfx[:, 2 * FD:3 * FD],
                        bass.AP(xin, (b * 6 + 2) * VPC, [[PLANE, P], [1, FD]]))
                    nc.sync.dma_start(
                        c3(bcdt[:, 0:3 * FD]),
                        bass.AP(xin, b * 6 * VPC + 3 * VPC,
                                [[PLANE, P], [VPC, 3], [1, FD]]))
                s(loads)

                # ---- A phase ----
                s(lambda: V.tensor_tensor(s1[:], adfx[:, 0:FD], adfx[:, FD:2 * FD], OP.add))
                s(lambda: V.tensor_tensor(s1[:], s1[:], adfx[:, 2 * FD:3 * FD], OP.add))
                s(lambda: V.tensor_scalar(q[:], s1[:], 1.0 / 3.0, None, OP.mult))
                s(lambda: V.tensor_tensor(Dt3, c3(adfx[:, 0:3 * FD]), _b3(q[:]), OP.subtract))
                # off-critical products on Pool
                s(lambda: G.tensor_tensor(ad[:], bcdt[:, 3 * FD:4 * FD], bcdt[:, 4 * FD:5 * FD], OP.mult))
                s(lambda: G.tensor_tensor(ad[:], ad[:], bcdt[:, 5 * FD:6 * FD], OP.mult))
                s(lambda: G.tensor_tensor(adfx[:, 5 * FD:6 * FD], b_, c_, OP.mult))
                s(lambda: S.activation(sqD, bcdt[:, 3 * FD:6 * FD], AF.Square))
                s(lambda: S.activation(sqU, bcdt[:, 0:3 * FD], AF.Square))
                # (st, su) strided pair sums over the 6-wide TU
                pA = TU[:].rearrange("p (c f) -> p c f", c=6)
                s(lambda: V.tensor_tensor(c2v(stsu[:]), pA[:, 0:6:3, :],
                                          pA[:, 1:6:3, :], OP.add))
                s(lambda: V.tensor_tensor(c2v(stsu[:]), c2v(stsu[:]),
                                          pA[:, 2:6:3, :], OP.add))
                s(lambda: V.tensor_scalar(su2[:], su, 2.0, None, OP.mult))
                s(lambda: V.tensor_tensor(p2[:], st, su2[:], OP.add))
                s(lambda: S.activation(pt[:], p2[:], AF.Sqrt, scale=1.0 / 6.0))
                s(lambda: V._custom_dve(
                    RECIPROCAL_APPROX_FAST, out=ip[:], in0=pt[:],
                    **RECIP_APPROX_FAST_CONSTS))
                # det cluster: critical parts on DVE
                s(lambda: V.tensor_tensor(c3(e1[:]), Dt3, sqU3[:, ::-1, :], OP.mult))
                s(lambda: V.tensor_tensor(dets[:], e1[:, 0:FD], e1[:, FD:2 * FD], OP.add))
                s(lambda: V.tensor_tensor(dets[:], dets[:], e1[:, 2 * FD:3 * FD], OP.add))
                # e1 dead; bcep/dm/det reuse its slices
                s(lambda: V.tensor_tensor(bcep, adfx[:, 5 * FD:6 * FD], e_, OP.mult))
                s(lambda: V.tensor_tensor(dm, dets[:], ad[:], OP.subtract))
                s(lambda: V._custom_dve(
                    SCALE_SUBMAX, out=det, in0=bcep, in1=dm, s0=2.0, s1=-3.0e38))
                # Pool: remaining off-critical A work
                cb_ap = bcdt[:, 0:2 * FD].rearrange("p (c f) -> p c f", c=2)[:, ::-1, :]
                e_b2 = e_.unsqueeze(1).broadcast_to((P, 2, FD))
                s(lambda: G.tensor_tensor(c2v(adfx[:, 3 * FD:5 * FD]), cb_ap, e_b2, OP.mult))
                s(lambda: G.tensor_tensor(c3(adfx[:, 0:3 * FD]), _b3(su), sqU3[:, ::-1, :],
                                          OP.subtract))  # w3 -> adf tag
                w3 = adfx
                # Pool mid-window precomputes (ready early, needed post-sigma):
                # dwd = sqD + w3 (= D^2 diag + w3), into the w3/adf tag in place
                s(lambda: G.tensor_tensor(c3(adfx[:, 0:3 * FD]), c3(sqD), c3(adfx[:, 0:3 * FD]), OP.add))
                dwd = w3

                # ---- serial chain ----
                s(lambda: V._custom_dve(
                    DETC_CLAMP, out=r, in0=det, in1=ip[:], s0=0.5, s1=-CL, imm2=CL))
                # r2..pcps in FD-halves: with the chunk-pair interleave this
                # gives 4-deep pipelining across the DVE<->ACT ping-pong.
                HF = FD // 2

                def half_chain(h):
                    a_, z_ = h * HF, (h + 1) * HF
                    s(lambda: S.activation(r2[:, a_:z_], r[:, a_:z_], AF.Square))
                    s(lambda: S.activation(sq[:, a_:z_], r2[:, a_:z_], AF.Sqrt,
                                           scale=-1.0, bias=1.0))
                    s(lambda: V._custom_dve(
                        DIV1, out=tq[:, a_:z_], in0=r[:, a_:z_], in1=sq[:, a_:z_],
                        s0=_D_C0, s1=_D_C1, imm2=0.0))
                    s(lambda: S.activation(at[:, a_:z_], tq[:, a_:z_], AF.Arctan))
                    s(lambda: S.activation(
                        sfcf[:, a_:z_], at[:, a_:z_], AF.Sin,
                        scale=-1.0 / 3.0, bias=PI6))
                    s(lambda: S.activation(
                        sfcf[:, FD + a_:FD + z_], at[:, a_:z_], AF.Sin,
                        scale=-1.0 / 3.0, bias=PI6 + math.pi / 2.0))
                    s(lambda: V.tensor_tensor(
                        c2v(pcps[:])[:, :, a_:z_],
                        pt[:, a_:z_].unsqueeze(1).broadcast_to((P, 2, HF)),
                        c2v(sfcf[:])[:, :, a_:z_], OP.mult))
                half_chain(0)
                half_chain(1)
                s(lambda: V.tensor_scalar(uut[:], ps, S3, None, OP.mult))
                s(lambda: V.tensor_tensor(uu[:], uut[:], pc, OP.add))
                # pg = (pc2 | g1a) packed in the pcps tile (dead after pc2/uut
                # reads); enables 2-wide fused (w1|g1b) and (w|g1c) below
                pg = pcps
                pc2v = pg[:, 0:FD]
                g1av = pg[:, FD:2 * FD]
                # d12 must read ps BEFORE pc2 overwrites the ps slot (pg alias)
                s(lambda: V.tensor_scalar(d12, ps, 2.0 * S3, None, OP.mult))
                s(lambda: V.tensor_scalar(pc2v, pc, 2.0, None, OP.mult))
                s(lambda: V.tensor_tensor(l1, q[:], uu[:], OP.subtract))
                s(lambda: V.tensor_tensor(l3, q[:], pc2v, OP.add))
                s(lambda: V.tensor_tensor(l2, l1, d12, OP.add))
                s(lambda: V.tensor_tensor(d13, l3, l1, OP.subtract))
                s(lambda: V.tensor_tensor(d23, d13, d12, OP.subtract))
                s(lambda: S.activation(lg[:, 0:FD], L[:, 0:FD], AF.Ln))
                s(lambda: S.activation(lg[:, FD:3 * FD], L[:, FD:3 * FD], AF.Ln))
                # bd = bce*Drev (e1 tag, free after tq); cross' = cross - bd.
                # DVE cols [0:512] fill the ln-wait gap; Pool (idle mid-window)
                # takes [512:1024].
                HS = 512
                s(lambda: G.tensor_tensor(c3(e1[:])[:, :, HS:FD],
                                          c3(bcdt[:, 0:3 * FD])[:, :, HS:FD],
                                          Dt3[:, ::-1, HS:FD], OP.mult))
                s(lambda: V.tensor_tensor(c3(e1[:])[:, :, 0:HS],
                                          c3(bcdt[:, 0:3 * FD])[:, :, 0:HS],
                                          Dt3[:, ::-1, 0:HS], OP.mult))
                s(lambda: G.tensor_tensor(c3(adfx[:, 3 * FD:6 * FD])[:, :, HS:FD],
                                          c3(adfx[:, 3 * FD:6 * FD])[:, :, HS:FD],
                                          c3(e1[:])[:, :, HS:FD], OP.subtract))
                s(lambda: V.tensor_tensor(c3(adfx[:, 3 * FD:6 * FD])[:, :, 0:HS],
                                          c3(adfx[:, 3 * FD:6 * FD])[:, :, 0:HS],
                                          c3(e1[:])[:, :, 0:HS], OP.subtract))
                # fill the ln-wait gap: g1a = 2pc - uu
                s(lambda: V.tensor_tensor(g1av, pc2v, uu[:], OP.subtract))
                s(lambda: V.tensor_tensor(
                    gg[:, 0:FD], lg[:, FD:2 * FD], lg[:, 0:FD], OP.subtract))
                s(lambda: V.tensor_tensor(
                    gg[:, FD:2 * FD], lg[:, 2 * FD:3 * FD], lg[:, FD:2 * FD],
                    OP.subtract))
                s(lambda: V._custom_dve(
                    DIV1M, out=c1f[:], in0=gg[:], in1=DD[:, 0:2 * FD],
                    s0=_D_C0, s1=_D_C1, imm2=ETA))
                c1 = c1f[:, 0:FD]
                s(lambda: V.tensor_tensor(c2s, c1f[:, FD:2 * FD], c1, OP.subtract))
                s(lambda: V._custom_dve(
                    DIV1M, out=c2[:], in0=c2s, in1=d13, s0=_D_C0, s1=_D_C1, imm2=ETA))
                # gamma = lg1 + uu*(c1 + c2*(2pc - uu))  (no sigma dependency)
                # (w1|g1b) = c2 * (pc2|g1a); (w|g1c) = that + c1  [2-wide fused]
                vw = gg
                s(lambda: V.tensor_tensor(
                    c2v(vw[:]), c2[:].unsqueeze(1).broadcast_to((P, 2, FD)),
                    c2v(pg[:]), OP.mult))
                s(lambda: V.tensor_tensor(
                    c2v(vw[:]), c2v(vw[:]),
                    c1.unsqueeze(1).broadcast_to((P, 2, FD)), OP.add))
                s(lambda: V.tensor_tensor(gam[:], uu[:], vw[:, FD:2 * FD], OP.mult))
                s(lambda: V.tensor_tensor(gam[:], gam[:], lg[:, 0:FD], OP.add))
                # w = sigma*c2 = c1 + c2*2pc  (w-route: no divide)
                # ---- assembly (each op split DVE/Pool by free columns) ----
                SPL = 824

                def bsl(ap_fd, a_, z_):
                    return ap_fd[:, a_:z_].unsqueeze(1).broadcast_to((P, 3, z_ - a_))

                def wide(dst3, in0f, in1f, op, b3idx):
                    # b3idx: which operand (0/1) is a broadcast [P,FD] scalar
                    def dve():
                        i0 = bsl(in0f, 0, SPL) if b3idx == 0 else in0f[:, :, 0:SPL]
                        i1 = bsl(in1f, 0, SPL) if b3idx == 1 else in1f[:, :, 0:SPL]
                        V.tensor_tensor(dst3[:, :, 0:SPL], i0, i1, op)

                    def poolp():
                        i0 = bsl(in0f, SPL, FD) if b3idx == 0 else in0f[:, :, SPL:FD]
                        i1 = bsl(in1f, SPL, FD) if b3idx == 1 else in1f[:, :, SPL:FD]
                        G.tensor_tensor(dst3[:, :, SPL:FD], i0, i1, op)
                    s(poolp)
                    s(dve)

                cross3 = c3(adfx[:, 3 * FD:6 * FD])
                e13 = c3(e1[:])
                dwd3 = c3(adfx[:, 0:3 * FD])
                bce3 = c3(bcdt[:, 0:3 * FD])
                # (om1|ym2) = w * (bce|Dt) as ONE 6-wide in-place op
                bcdt6 = bcdt[:].rearrange("p (c f) -> p c f", c=6)

                def b6sl(ap_fd, a_, z_):
                    return ap_fd[:, a_:z_].unsqueeze(1).broadcast_to((P, 6, z_ - a_))
                s(lambda: G.tensor_tensor(bcdt6[:, :, SPL:FD],
                                          b6sl(vw[:, 0:FD], SPL, FD),
                                          bcdt6[:, :, SPL:FD], OP.mult))
                s(lambda: V.tensor_tensor(bcdt6[:, :, 0:SPL],
                                          b6sl(vw[:, 0:FD], 0, SPL),
                                          bcdt6[:, :, 0:SPL], OP.mult))
                # (ym1|om2) = c2 * (dwd|cross') as ONE 6-wide op into TU
                adfx6 = adfx[:].rearrange("p (c f) -> p c f", c=6)
                TU6 = TU[:].rearrange("p (c f) -> p c f", c=6)
                s(lambda: G.tensor_tensor(TU6[:, :, SPL:FD],
                                          b6sl(c2[:], SPL, FD),
                                          adfx6[:, :, SPL:FD], OP.mult))
                s(lambda: V.tensor_tensor(TU6[:, :, 0:SPL],
                                          b6sl(c2[:], 0, SPL),
                                          adfx6[:, :, 0:SPL], OP.mult))
                wide(Yo3, Yo3, bce3, OP.add, -1)

                def store_o():
                    nc.sync.dma_start(
                        bass.AP(yout, b * 6 * VPC + 3 * VPC,
                                [[PLANE, P], [VPC, 3], [1, SPL]]),
                        Yo3[:, :, 0:SPL])
                    nc.sync.dma_start(
                        bass.AP(yout, b * 6 * VPC + 3 * VPC + SPL,
                                [[PLANE, P], [VPC, 3], [1, FD - SPL]]),
                        Yo3[:, :, SPL:FD])
                s(store_o)
                # Yd: ym1 already in Yd3 from the fused op; add w*Dt
                wide(Yd3, Yd3, Dt3, OP.add, -1)
                # final add sub-split so stores start before the whole row is done
                HSP = 330

                def dseg(a_, z_, eng):
                    s(lambda: eng.tensor_tensor(Yd3[:, :, a_:z_], Yd3[:, :, a_:z_],
                                                bsl(gam[:], a_, z_), OP.add))
                    s(lambda: nc.sync.dma_start(
                        bass.AP(yout, b * 6 * VPC + a_,
                                [[PLANE, P], [VPC, 3], [1, z_ - a_]]),
                        Yd3[:, :, a_:z_]))
                dseg(SPL, FD, G)
                dseg(0, HSP, V)
                dseg(HSP, SPL, V)
                return st_

            lists = [chunk_steps(ci) for ci in range(NCHUNK)]
            n = max(len(x) for x in lists)
            for i in range(n):
                for ls in lists:
                    if i < len(ls):
                        ls[i]()
    nc.finalize()
    return nc


def kernel(x):
    x = np.asarray(x)
    xf = x.reshape(B, 9, NV).astype(np.float16)
    sel = [0, 4, 8, 1, 2, 5]  # a d f b c e
    in_maps = []
    for k in range(NCORE):
        shard = np.ascontiguousarray(xf[:, sel, k * VPC:(k + 1) * VPC])
        in_maps.append({"xin": shard})
    if "nc" not in _CACHE:
        _CACHE["nc"] = build()
    res = run_bass_kernel_spmd(_CACHE["nc"], in_maps, core_ids=list(range(NCORE)))
    out = np.empty((B, 9, NV), np.float32)
    # device channel order: (Y00, Y11, Y22, Y01, Y02, Y12)
    dst = [0, 4, 8, 1, 2, 5]
    for k in range(NCORE):
        out[:, dst, k * VPC:(k + 1) * VPC] = res.results[k]["yout"].astype(np.float32)
    out[:, 3] = out[:, 1]
    out[:, 6] = out[:, 2]
    out[:, 7] = out[:, 5]
    return out.reshape(x.shape)
